# revision 12
# baseline (speedup 1.0000x reference)
"""Causal self-attention (softmax over the QUERY axis) for Trainium2, 8 cores.

Reference semantics (B=2, S=2048, D=1024, H=16, HD=64):
    q = x @ Wq; k = x @ Wk; v = x @ Wv          (per batch)
    s[b,h,q,k] = <q_bqh, k_bkh>;  mask k > q -> -inf
    w = softmax(s / sqrt(1024), axis=q)          # normalize over QUERY axis
    ctx[b,q,h,:] = sum_k w[b,h,q,k] * v[b,k,h,:]

Sharding: core c handles batch b = c // 4 and head group g = c % 4
(4 heads: 4g..4g+3).  Per core everything is done in a transposed
score layout S^T[k, q], which makes the query-axis softmax a FREE-AXIS
reduction, and the 1/Z[k] normalizer folds into V rows (no per-element
divide): ctx[q,d] = sum_k exp(s)/Z[k] * v[k,d] = sum_k exp(s) * (v[k,d]/Z[k]).

Scheduling (v3):
  - Inputs are HOST-PRE-ARRANGED so every DMA moves contiguous per-partition
    blocks (8KB xT chunks, 4KB weight slices) at line rate; xT rides the SP
    ring, weights ride the ACT ring (descriptor-gen only, finished before the
    first exp), outputs trickle out on SP.
  - The exp table is prewarmed by a dummy activation at t~0.
  - The two heads of a pair sit on disjoint PE-array row halves (partitions
    0-63 / 64-127); their score matmuls are emitted interleaved and overlap
    in the array (same trick ctx_pair_packed uses on column halves).
  - Phase 0 = pair-0 scores (rows descending) with q/k/v projections of
    pair 0 as PE filler.  Phase 1 = pair-1 scores with pair-1 q/k
    projections (per-qc, just-in-time), pair-0 ctx chains (qc3..qc0, which
    frees E slots in exactly the order the descending rows need), and the
    EARLY part of pair-1 ctx chains as filler.
  - ctx accumulation order within a chain is coverage-correct but starts
    with kt=4qc (which spans the whole 512-col window), so a chain's high-kt
    matmuls can run long before rows 3..0 exist; only each chain's kt 3..0
    matmuls sit behind the last exp -> short tail.

Device layouts (per core):
    xT  host-arranged [128, 4 qc, 8 c, 512] bf16
    Wq/Wk/Wv host-arranged [128, 8 c, 256] bf16
    qT/kT  [128(2 heads x 64), 2 pairs, 2048] bf16 (projection output)
    v      [128(s in tile), 16 kt, 256(4 heads x 64)] bf16
    E      exp(scores^T) row kt: [128, 2048-128kt] bf16 per head
    out    [256(4 heads x 64), 2048] f32 = ctx^T; host transposes back.
"""

import numpy as np
import ml_dtypes
from contextlib import ExitStack

import concourse.bass as bass
import concourse.tile as tile
from concourse import bacc, mybir
from concourse.bass_utils import run_bass_kernel_spmd

BF16 = mybir.dt.bfloat16
F32 = mybir.dt.float32

B, S, D, H, HD = 2, 2048, 1024, 16, 64
NCORES = 8
HL = 4                       # heads per core
KC = D // 128                # 8 contraction chunks
KT = S // 128                # 16 key tiles
QC = S // 512                # 4 query chunks of 512
SCALE = 1.0 / float(np.sqrt(np.float32(D)))   # 1/32


def _ebufs(kt):
    # slots per E tag: all four heads' rows 8-15 coexist (4 slots), so
    # pair-1's descending rows 15..8 never gate on pair-0's ctx chains
    # (ACT never starves at the pair boundary); rows 0-7 stay at 2 and
    # mesh with the chain emission order.
    return 4 if kt >= 8 else 2


def _emit(ctx: ExitStack, tc: tile.TileContext, out_ap, xT, wq, wk, wv):
    nc = tc.nc
    Exp = mybir.ActivationFunctionType.Exp

    consts = ctx.enter_context(tc.tile_pool(name="consts", bufs=1))
    qkp = ctx.enter_context(tc.tile_pool(name="qk", bufs=1))
    vp = ctx.enter_context(tc.tile_pool(name="v", bufs=1))
    epool = ctx.enter_context(tc.tile_pool(name="e", bufs=2))
    zpool = ctx.enter_context(tc.tile_pool(name="z", bufs=4))
    spool = ctx.enter_context(tc.tile_pool(name="scr", bufs=4))
    outp = ctx.enter_context(tc.tile_pool(name="outp", bufs=1))
    # scores rows: [128, 1536] = 3 banks x 2 bufs = 6 banks; projections and
    # ctx accumulations share one 2-slot [*, 512] pool (2 banks).
    sc_ps = ctx.enter_context(tc.tile_pool(name="sc_ps", bufs=2, space="PSUM"))
    small_ps = ctx.enter_context(tc.tile_pool(name="small_ps", bufs=2, space="PSUM"))

    # ---- input DMA: contiguous per-partition blocks on both HWDGE rings.
    # The first-needed chunk (xT qc3) is split across sync+scalar so it
    # lands in half the time; later chunks stream behind it in need order.
    xT_cs = [None] * 4
    for sc in (3, 2, 1, 0):
        xT_cs[sc] = consts.tile([128, KC, 512], BF16, tag=f"xT{sc}",
                                name=f"xT{sc}_sb")
    w_sb = {}
    for name, t in (("q", wq), ("k", wk), ("v", wv)):
        w_sb[name] = consts.tile([128, KC, HL * HD], BF16, tag=f"w{name}",
                                 name=f"w{name}_sb")
    x3 = xT[:, 4096 * 3:4096 * 4].rearrange("p (c j) -> p c j", c=KC)
    nc.sync.dma_start(out=xT_cs[3][:, 0:4, :], in_=x3[:, 0:4, :])
    nc.scalar.dma_start(out=w_sb["q"],
                        in_=wq.rearrange("p (c n) -> p c n", c=KC))
    nc.scalar.dma_start(out=xT_cs[3][:, 4:KC, :], in_=x3[:, 4:KC, :])
    nc.scalar.dma_start(out=w_sb["k"],
                        in_=wk.rearrange("p (c n) -> p c n", c=KC))
    nc.scalar.dma_start(out=w_sb["v"],
                        in_=wv.rearrange("p (c n) -> p c n", c=KC))
    for sc in (2, 1, 0):
        nc.sync.dma_start(
            out=xT_cs[sc],
            in_=xT[:, 4096 * sc:4096 * (sc + 1)].rearrange(
                "p (c j) -> p c j", c=KC))

    def xT_slice(c, lo, w):
        sc, o = divmod(lo, 512)
        assert o + w <= 512
        return xT_cs[sc][:, c, o:o + w]

    qT_sb = qkp.tile([128, 2, S], BF16, tag="qT")
    kT_sb = qkp.tile([128, 2, S], BF16, tag="kT")
    v_sb = vp.tile([128, KT, HL * HD], BF16, tag="v")
    v2_sb = vp.tile([128, KT, HL * HD], BF16, tag="v2")
    out_sb = outp.tile([128, 2, S], F32, tag="out")

    def proj_chain(name, pair, qc):
        dst = qT_sb if name == "q" else kT_sb
        ps = small_ps.tile([128, 512], F32, tag="ps512", name="pps")
        for c in range(KC):
            nc.tensor.matmul(
                ps,
                w_sb[name][:, c, 128 * pair:128 * pair + 128],
                xT_cs[qc][:, c, :],
                start=(c == 0), stop=(c == KC - 1),
            )
        nc.vector.tensor_copy(dst[:, pair, 512 * qc:512 * qc + 512], ps)

    def proj_v(st):
        # v natural layout: out partitions = s-within-tile, cols = 4 heads x 64
        ps = small_ps.tile([128, HL * HD], F32, tag="ps512", name="pps")
        for c in range(KC):
            nc.tensor.matmul(
                ps,
                xT_slice(c, 128 * st, 128),
                w_sb["v"][:, c, :],
                start=(c == 0), stop=(c == KC - 1),
            )
        nc.vector.tensor_copy(v_sb[:, st, :], ps)

    def alloc_head(h):
        zp = zpool.tile([128, KT, 2], F32, tag="zp", name=f"zp{h}")
        inv = zpool.tile([128, KT], F32, tag="inv", name=f"inv{h}")
        nc.vector.memset(zp, 0.0)
        nc.vector.memset(inv, 0.0)
        return {"zp": zp, "inv": inv, "e": [None] * KT, "h": h}

    def _diag_fix(st, kt, e_row, dve_z):
        """post-exp diagonal handling for one head's row kt."""
        diag = e_row[:, 0:128]
        if not dve_z:
            # gather the invalid part (its sum is subtracted from Z);
            # is_lt is unimplemented in walrus codegen, so use is_ge with
            # negated affine coefficients (j < p <=> p - j - 1 >= 0).
            scr = spool.tile([128, 128], BF16, tag="scr", name="scr")
            nc.gpsimd.affine_select(
                scr, diag, pattern=[[-1, 128]],
                compare_op=mybir.AluOpType.is_ge, fill=0.0,
                base=-1, channel_multiplier=1,
            )
            nc.vector.tensor_reduce(
                st["inv"][:, kt:kt + 1], scr,
                axis=mybir.AxisListType.X, op=mybir.AluOpType.add,
            )
        nc.gpsimd.affine_select(
            diag, diag, pattern=[[1, 128]],
            compare_op=mybir.AluOpType.is_ge, fill=0.0,
            base=0, channel_multiplier=-1,
        )
        if dve_z:
            # post-zero row sum is exactly the valid Z contribution
            W = S - 128 * kt
            nc.vector.tensor_reduce(
                st["zp"][:, kt, 0:1], e_row[:, 0:W],
                axis=mybir.AxisListType.X, op=mybir.AluOpType.add,
            )

    def score_row_pair(sta, stb, kt):
        """scores^T row kt for BOTH heads of a pair, matmuls interleaved so
        the two [64,128] stationaries (partition halves 0-63 / 64-127) can
        overlap in the PE array."""
        pair = sta["h"] // 2
        q0k = 128 * kt
        W = S - q0k
        tiles = [(q0k, min(W, 1536))]
        if W > 1536:
            tiles.append((q0k + 1536, W - 1536))
        dve_z = kt >= 8    # short rows: Z via DVE post-zero sum (ACT stays hot)
        e_rows = {}
        for half, st in ((0, sta), (1, stb)):
            e_rows[half] = epool.tile(
                [128, W], BF16, tag=f"E{kt}", name=f"e{kt}h{st['h']}",
                bufs=_ebufs(kt))
            st["e"][kt] = e_rows[half]
        for ti, (lo, w) in enumerate(tiles):
            pss = {}
            for half, st in ((0, sta), (1, stb)):
                pss[half] = sc_ps.tile([128, w], F32, tag="sc", name="scps")
            # interleave the two heads' matmul chunks: disjoint row groups
            c0 = 0
            while c0 < w:
                c1 = min(w, c0 + 512)
                for half in (0, 1):
                    pb = 64 * half
                    nc.tensor.matmul(
                        pss[half][:, c0:c1],
                        kT_sb[pb:pb + 64, pair, q0k:q0k + 128],
                        qT_sb[pb:pb + 64, pair, lo + c0:lo + c1],
                        start=True, stop=True,
                    )
                c0 = c1
            for half, st in ((0, sta), (1, stb)):
                if dve_z or ti == 1:
                    # Z contribution via DVE row-sum (second chunks are
                    # <=512 wide; cheap) -- saves an ACT READ_ACCUMULATOR.
                    nc.scalar.activation(
                        e_rows[half][:, lo - q0k:lo - q0k + w], pss[half][:, 0:w],
                        Exp, scale=SCALE,
                    )
                    if not dve_z:
                        nc.vector.tensor_reduce(
                            st["zp"][:, kt, 1:2],
                            e_rows[half][:, lo - q0k:lo - q0k + w],
                            axis=mybir.AxisListType.X, op=mybir.AluOpType.add,
                        )
                else:
                    nc.scalar.activation(
                        e_rows[half][:, lo - q0k:lo - q0k + w], pss[half][:, 0:w],
                        Exp, scale=SCALE,
                        accum_out=st["zp"][:, kt, ti:ti + 1],
                    )
        for half, st in ((0, sta), (1, stb)):
            _diag_fix(st, kt, e_rows[half], dve_z)

    def z_v2(st, k0, k1):
        """finalize Z for rows [k0, k1) and scale V rows by 1/Z."""
        h = st["h"]
        n = k1 - k0
        zs = zpool.tile([128, n], F32, tag="zs", name="zs")
        nc.vector.tensor_reduce(zs, st["zp"][:, k0:k1, :],
                                axis=mybir.AxisListType.X,
                                op=mybir.AluOpType.add)
        zv = zpool.tile([128, n], F32, tag="zv", name="zv")
        nc.vector.tensor_sub(zv, zs, st["inv"][:, k0:k1])
        zi = zpool.tile([128, n], F32, tag="zi", name="zi")
        nc.vector.reciprocal(zi, zv)
        zia = zi[:, :]
        zi_bc = bass.AP(tensor=zia.tensor, offset=zia.offset,
                        ap=[zia.ap[0], zia.ap[1], [0, HD]])
        nc.vector.tensor_mul(
            v2_sb[:, k0:k1, HD * h:HD * h + HD],
            v_sb[:, k0:k1, HD * h:HD * h + HD],
            zi_bc,
        )

    def ctx_begin(sta, stb, qc):
        ps = small_ps.tile([128, 512], F32, tag="ps512", name="cpp")
        return {"a": sta, "b": stb, "qc": qc, "ps": ps,
                "n": 2 * (4 * qc + 4), "done": 0, "first_kt": None}

    def ctx_mms(ch, kts):
        """col-packed ctx matmuls for the given kt subset.  The FIRST call's
        first kt must cover the whole 512-col window (any kt <= 4qc does).
        start/stop must be set on BOTH halves' first/last matmuls (the two
        halves write disjoint psum partition ranges)."""
        qc = ch["qc"]
        if ch["first_kt"] is None:
            ch["first_kt"] = kts[0]
        for kt in kts:
            q0 = max(512 * qc, 128 * kt)
            w = 512 * qc + 512 - q0
            for half, st in ((0, ch["a"]), (1, ch["b"])):
                h = st["h"]
                rhs = st["e"][kt][:, q0 - 128 * kt:q0 - 128 * kt + w]
                ch["done"] += 1
                nc.tensor.matmul(
                    ch["ps"][64 * half:64 * half + 64, q0 - 512 * qc:512],
                    v2_sb[:, kt, HD * h:HD * h + HD],
                    rhs,
                    start=(kt == ch["first_kt"]),
                    stop=(ch["done"] >= ch["n"] - 1),
                    tile_position=(0, 64 * half),
                    skip_group_check=True,
                )
        assert ch["done"] <= ch["n"]

    def ctx_finish(ch):
        assert ch["done"] == ch["n"]
        pair = ch["a"]["h"] // 2
        qc = ch["qc"]
        nc.vector.tensor_copy(out_sb[:, pair, 512 * qc:512 * qc + 512],
                              ch["ps"])
        nc.sync.dma_start(
            out=out_ap[128 * pair:128 * pair + 128, 512 * qc:512 * qc + 512],
            in_=out_sb[:, pair, 512 * qc:512 * qc + 512],
        )

    def ctx_full(sta, stb, qc):
        ch = ctx_begin(sta, stb, qc)
        ctx_mms(ch, range(4 * qc + 4))
        ctx_finish(ch)

    # ---- emission (order = scheduling priority) ----
    # phase 0: pair-0 scores descending, pair-0 q/k and v projections
    # interleaved as PE filler.
    st0 = alloc_head(0)
    st1 = alloc_head(1)
    for qc in (3, 2, 1, 0):
        proj_chain("q", 0, qc)
        proj_chain("k", 0, qc)
        for kt in range(4 * qc + 3, 4 * qc - 1, -1):
            score_row_pair(st0, st1, kt)
            proj_v(kt)         # fills the PE gap while ACT drains the row
        z_v2(st0, 4 * qc, 4 * qc + 4)
        z_v2(st1, 4 * qc, 4 * qc + 4)
    # phase 1: pair-1 scores descending with just-in-time pair-1 projections,
    # pair-0 ctx chains qc3..qc0 (frees E tags 0-7 in exactly the order the
    # descending rows reuse them), and early parts of pair-1 ctx as filler.
    st2 = alloc_head(2)
    st3 = alloc_head(3)
    proj_chain("q", 1, 3)
    proj_chain("k", 1, 3)
    for kt in (15, 14, 13, 12):
        score_row_pair(st2, st3, kt)
    z_v2(st2, 12, 16)
    z_v2(st3, 12, 16)
    proj_chain("q", 1, 2)
    proj_chain("k", 1, 2)
    for kt in (11, 10, 9, 8):
        score_row_pair(st2, st3, kt)
    z_v2(st2, 8, 12)
    z_v2(st3, 8, 12)
    proj_chain("q", 1, 1)
    proj_chain("k", 1, 1)
    # pair-0 ctx: by now ACT has drained all of pair-0 (its exps precede
    # every pair-1 exp in the ACT queue), so these chains run stall-free;
    # they also free E tags 4-7 / 0-3 for the remaining descending rows.
    ctx_full(st0, st1, 3)
    ctx_full(st0, st1, 2)
    ctx_full(st0, st1, 1)
    for kt in (7, 6):
        score_row_pair(st2, st3, kt)
        proj_chain("q" if kt == 7 else "k", 1, 0)   # filler between rows
    for kt in (5, 4):
        score_row_pair(st2, st3, kt)
    z_v2(st2, 4, 8)
    z_v2(st3, 4, 8)
    ctx_full(st0, st1, 0)
    # pair-1 ctx chains qc3/qc2: everything except rows 3..0 runs as filler
    # interleaved with the last rows (kt=4qc first: it covers the whole
    # window, so start=True is coverage-correct).
    ch3 = ctx_begin(st2, st3, 3)
    ch2 = ctx_begin(st2, st3, 2)
    score_row_pair(st2, st3, 3)
    ctx_mms(ch3, [12, 13, 14, 15, 11, 10])
    score_row_pair(st2, st3, 2)
    ctx_mms(ch3, [9, 8, 7, 6, 5, 4])
    score_row_pair(st2, st3, 1)
    ctx_mms(ch2, [8, 9, 10, 11])
    score_row_pair(st2, st3, 0)
    ctx_mms(ch2, [7, 6, 5, 4])
    z_v2(st2, 0, 4)
    z_v2(st3, 0, 4)
    # tail: only kt 3..0 matmuls + the short low chains remain.  ch3/ch2
    # must finish (and free their psum slots) before ch1/ch0 can begin.
    ctx_mms(ch3, [3, 2, 1, 0])
    ctx_finish(ch3)
    ctx_mms(ch2, [3, 2, 1, 0])
    ctx_finish(ch2)
    ch1 = ctx_begin(st2, st3, 1)
    ctx_mms(ch1, [4, 5, 6, 7, 3, 2, 1, 0])
    ctx_finish(ch1)
    ctx_full(st2, st3, 0)


_PROG = None


def _build_program():
    global _PROG
    if _PROG is not None:
        return _PROG
    nc = bacc.Bacc("TRN2", target_bir_lowering=False, debug=False,
                   num_devices=NCORES)
    xT = nc.dram_tensor("xT", [128, 4 * KC * 512], BF16, kind="ExternalInput").ap()
    wq = nc.dram_tensor("wq", [128, KC * HL * HD], BF16, kind="ExternalInput").ap()
    wk = nc.dram_tensor("wk", [128, KC * HL * HD], BF16, kind="ExternalInput").ap()
    wv = nc.dram_tensor("wv", [128, KC * HL * HD], BF16, kind="ExternalInput").ap()
    out = nc.dram_tensor("out", [HL * HD, S], F32, kind="ExternalOutput").ap()
    with tile.TileContext(nc) as tc:
        with ExitStack() as stack:
            _emit(stack, tc, out, xT, wq, wk, wv)
    nc.compile()
    _PROG = nc
    return nc


def _arrange_xT(xb):
    # [S, D] batch slice -> xT[d, s] -> [p, qc, c, j] with d = c*128+p,
    # s = 512*qc + j -> flattened [128, 4*8*512]; every DMA chunk
    # (one qc) is then contiguous per partition.
    xT = np.asarray(xb).T                              # [D, S]
    t = xT.reshape(KC, 128, QC, 512).transpose(1, 2, 0, 3)
    return np.ascontiguousarray(t.reshape(128, QC * KC * 512))


def _arrange_w(w, cols):
    # [D, 256] slice -> [p, c, n] with d = c*128+p -> [128, 8*256]
    t = np.asarray(w)[:, cols].reshape(KC, 128, HL * HD).transpose(1, 0, 2)
    return np.ascontiguousarray(t.reshape(128, KC * HL * HD))


def make_in_maps(x, Wq, Wk, Wv):
    bf = ml_dtypes.bfloat16
    in_maps = []
    for core in range(NCORES):
        b, g = divmod(core, NCORES // B)
        cols = slice(HL * HD * g, HL * HD * (g + 1))
        in_maps.append({
            "xT": _arrange_xT(x[b]).astype(bf),
            "wq": _arrange_w(Wq, cols).astype(bf),
            "wk": _arrange_w(Wk, cols).astype(bf),
            "wv": _arrange_w(Wv, cols).astype(bf),
        })
    return in_maps


def assemble(results):
    out = np.empty((B, S, H * HD), np.float32)
    for core in range(NCORES):
        b, g = divmod(core, NCORES // B)
        out[b, :, HL * HD * g:HL * HD * (g + 1)] = results[core]["out"].T
    return out


def kernel(**inputs):
    nc = _build_program()
    in_maps = make_in_maps(inputs["x"], inputs["Wq"], inputs["Wk"], inputs["Wv"])
    res = run_bass_kernel_spmd(nc, in_maps, list(range(NCORES)))
    return assemble(res.results)


# revision 13
# speedup vs baseline: 1.1491x; 1.1491x over previous
"""Causal self-attention (softmax over the QUERY axis) for Trainium2, 8 cores.

Reference semantics (B=2, S=2048, D=1024, H=16, HD=64):
    q = x @ Wq; k = x @ Wk; v = x @ Wv          (per batch)
    s[b,h,q,k] = <q_bqh, k_bkh>;  mask k > q -> -inf
    w = softmax(s / sqrt(1024), axis=q)          # normalize over QUERY axis
    ctx[b,q,h,:] = sum_k w[b,h,q,k] * v[b,k,h,:]

Sharding: core c handles batch b = c // 4 and head group g = c % 4
(4 heads: 4g..4g+3).  Per core everything is done in a transposed
score layout S^T[k, q], which makes the query-axis softmax a FREE-AXIS
reduction, and the 1/Z[k] normalizer folds into V rows (no per-element
divide): ctx[q,d] = sum_k exp(s)/Z[k] * v[k,d] = sum_k exp(s) * (v[k,d]/Z[k]).

Scheduling (v3):
  - Inputs are HOST-PRE-ARRANGED so every DMA moves contiguous per-partition
    blocks (8KB xT chunks, 4KB weight slices) at line rate; xT rides the SP
    ring, weights ride the ACT ring (descriptor-gen only, finished before the
    first exp), outputs trickle out on SP.
  - The exp table is prewarmed by a dummy activation at t~0.
  - The two heads of a pair sit on disjoint PE-array row halves (partitions
    0-63 / 64-127); their score matmuls are emitted interleaved and overlap
    in the array (same trick ctx_pair_packed uses on column halves).
  - Phase 0 = pair-0 scores (rows descending) with q/k/v projections of
    pair 0 as PE filler.  Phase 1 = pair-1 scores with pair-1 q/k
    projections (per-qc, just-in-time), pair-0 ctx chains (qc3..qc0, which
    frees E slots in exactly the order the descending rows need), and the
    EARLY part of pair-1 ctx chains as filler.
  - ctx accumulation order within a chain is coverage-correct but starts
    with kt=4qc (which spans the whole 512-col window), so a chain's high-kt
    matmuls can run long before rows 3..0 exist; only each chain's kt 3..0
    matmuls sit behind the last exp -> short tail.

Device layouts (per core):
    xT  host-arranged [128, 4 qc, 8 c, 512] bf16
    Wq/Wk/Wv host-arranged [128, 8 c, 256] bf16
    qT/kT  [128(2 heads x 64), 2 pairs, 2048] bf16 (projection output)
    v      [128(s in tile), 16 kt, 256(4 heads x 64)] bf16
    E      exp(scores^T) row kt: [128, 2048-128kt] bf16 per head
    out    [256(4 heads x 64), 2048] f32 = ctx^T; host transposes back.
"""

import numpy as np
import ml_dtypes
from contextlib import ExitStack

import concourse.bass as bass
import concourse.tile as tile
from concourse import bacc, mybir
from concourse.bass_utils import run_bass_kernel_spmd

BF16 = mybir.dt.bfloat16
F32 = mybir.dt.float32

B, S, D, H, HD = 2, 2048, 1024, 16, 64
NCORES = 8
HL = 4                       # heads per core
KC = D // 128                # 8 contraction chunks
KT = S // 128                # 16 key tiles
QC = S // 512                # 4 query chunks of 512
SCALE = 1.0 / float(np.sqrt(np.float32(D)))   # 1/32


def _ebufs(kt):
    # slots per E tag: all four heads' rows 8-15 coexist (4 slots), so
    # pair-1's descending rows 15..8 never gate on pair-0's ctx chains
    # (ACT never starves at the pair boundary); rows 0-7 stay at 2 and
    # mesh with the chain emission order.
    return 4 if kt >= 8 else 2


def _emit(ctx: ExitStack, tc: tile.TileContext, out_ap, xT, wq, wk, wv):
    nc = tc.nc
    Exp = mybir.ActivationFunctionType.Exp

    consts = ctx.enter_context(tc.tile_pool(name="consts", bufs=1))
    qkp = ctx.enter_context(tc.tile_pool(name="qk", bufs=1))
    vp = ctx.enter_context(tc.tile_pool(name="v", bufs=1))
    epool = ctx.enter_context(tc.tile_pool(name="e", bufs=2))
    zpool = ctx.enter_context(tc.tile_pool(name="z", bufs=4))
    spool = ctx.enter_context(tc.tile_pool(name="scr", bufs=4))
    outp = ctx.enter_context(tc.tile_pool(name="outp", bufs=1))
    # scores rows: [128, 1536] = 3 banks x 2 bufs = 6 banks; projections and
    # ctx accumulations share one 2-slot [*, 512] pool (2 banks).
    sc_ps = ctx.enter_context(tc.tile_pool(name="sc_ps", bufs=2, space="PSUM"))
    small_ps = ctx.enter_context(tc.tile_pool(name="small_ps", bufs=2, space="PSUM"))

    # ---- input DMA: contiguous per-partition blocks on both HWDGE rings.
    # The first-needed chunk (xT qc3) is split across sync+scalar so it
    # lands in half the time; later chunks stream behind it in need order.
    xT_cs = [None] * 4
    for sc in (3, 2, 1, 0):
        xT_cs[sc] = consts.tile([128, KC, 512], BF16, tag=f"xT{sc}",
                                name=f"xT{sc}_sb")
    w_sb = {}
    for name, t in (("q", wq), ("k", wk), ("v", wv)):
        w_sb[name] = consts.tile([128, KC, HL * HD], BF16, tag=f"w{name}",
                                 name=f"w{name}_sb")
    x3 = xT[:, 4096 * 3:4096 * 4].rearrange("p (c j) -> p c j", c=KC)
    nc.sync.dma_start(out=xT_cs[3][:, 0:4, :], in_=x3[:, 0:4, :])
    nc.scalar.dma_start(out=w_sb["q"],
                        in_=wq.rearrange("p (c n) -> p c n", c=KC))
    nc.scalar.dma_start(out=xT_cs[3][:, 4:KC, :], in_=x3[:, 4:KC, :])
    nc.scalar.dma_start(out=w_sb["k"],
                        in_=wk.rearrange("p (c n) -> p c n", c=KC))
    nc.scalar.dma_start(out=w_sb["v"],
                        in_=wv.rearrange("p (c n) -> p c n", c=KC))
    for sc in (2, 1, 0):
        nc.sync.dma_start(
            out=xT_cs[sc],
            in_=xT[:, 4096 * sc:4096 * (sc + 1)].rearrange(
                "p (c j) -> p c j", c=KC))

    def xT_slice(c, lo, w):
        sc, o = divmod(lo, 512)
        assert o + w <= 512
        return xT_cs[sc][:, c, o:o + w]

    qT_sb = qkp.tile([128, 2, S], BF16, tag="qT")
    kT_sb = qkp.tile([128, 2, S], BF16, tag="kT")
    v_sb = vp.tile([128, KT, HL * HD], BF16, tag="v")
    v2_sb = vp.tile([128, KT, HL * HD], BF16, tag="v2")
    out_sb = outp.tile([128, 2, S], F32, tag="out")

    def proj_chain(name, pair, qc):
        dst = qT_sb if name == "q" else kT_sb
        ps = small_ps.tile([128, 512], F32, tag="ps512", name="pps")
        for c in range(KC):
            nc.tensor.matmul(
                ps,
                w_sb[name][:, c, 128 * pair:128 * pair + 128],
                xT_cs[qc][:, c, :],
                start=(c == 0), stop=(c == KC - 1),
            )
        nc.vector.tensor_copy(dst[:, pair, 512 * qc:512 * qc + 512], ps)

    def proj_v(st):
        # v natural layout: out partitions = s-within-tile, cols = 4 heads x 64
        ps = small_ps.tile([128, HL * HD], F32, tag="ps512", name="pps")
        for c in range(KC):
            nc.tensor.matmul(
                ps,
                xT_slice(c, 128 * st, 128),
                w_sb["v"][:, c, :],
                start=(c == 0), stop=(c == KC - 1),
            )
        nc.vector.tensor_copy(v_sb[:, st, :], ps)

    def alloc_head(h):
        zp = zpool.tile([128, KT, 2], F32, tag="zp", name=f"zp{h}")
        inv = zpool.tile([128, KT], F32, tag="inv", name=f"inv{h}")
        nc.vector.memset(zp, 0.0)
        nc.vector.memset(inv, 0.0)
        return {"zp": zp, "inv": inv, "e": [None] * KT, "h": h}

    def _diag_fix(st, kt, e_row, dve_z):
        """post-exp diagonal handling for one head's row kt."""
        diag = e_row[:, 0:128]
        if not dve_z:
            # gather the invalid part (its sum is subtracted from Z);
            # is_lt is unimplemented in walrus codegen, so use is_ge with
            # negated affine coefficients (j < p <=> p - j - 1 >= 0).
            scr = spool.tile([128, 128], BF16, tag="scr", name="scr")
            nc.gpsimd.affine_select(
                scr, diag, pattern=[[-1, 128]],
                compare_op=mybir.AluOpType.is_ge, fill=0.0,
                base=-1, channel_multiplier=1,
            )
            nc.vector.tensor_reduce(
                st["inv"][:, kt:kt + 1], scr,
                axis=mybir.AxisListType.X, op=mybir.AluOpType.add,
            )
        nc.gpsimd.affine_select(
            diag, diag, pattern=[[1, 128]],
            compare_op=mybir.AluOpType.is_ge, fill=0.0,
            base=0, channel_multiplier=-1,
        )
        if dve_z:
            # post-zero row sum is exactly the valid Z contribution
            W = S - 128 * kt
            nc.vector.tensor_reduce(
                st["zp"][:, kt, 0:1], e_row[:, 0:W],
                axis=mybir.AxisListType.X, op=mybir.AluOpType.add,
            )

    def score_row_pair(sta, stb, kt):
        """scores^T row kt for BOTH heads of a pair, matmuls interleaved so
        the two [64,128] stationaries (partition halves 0-63 / 64-127) can
        overlap in the PE array."""
        pair = sta["h"] // 2
        q0k = 128 * kt
        W = S - q0k
        tiles = [(q0k, min(W, 1536))]
        if W > 1536:
            tiles.append((q0k + 1536, W - 1536))
        dve_z = kt >= 8    # short rows: Z via DVE post-zero sum (ACT stays hot)
        e_rows = {}
        for half, st in ((0, sta), (1, stb)):
            e_rows[half] = epool.tile(
                [128, W], BF16, tag=f"E{kt}", name=f"e{kt}h{st['h']}",
                bufs=_ebufs(kt))
            st["e"][kt] = e_rows[half]
        for ti, (lo, w) in enumerate(tiles):
            pss = {}
            for half, st in ((0, sta), (1, stb)):
                pss[half] = sc_ps.tile([128, w], F32, tag="sc", name="scps")
            # interleave the two heads' matmul chunks: disjoint row groups
            c0 = 0
            while c0 < w:
                c1 = min(w, c0 + 512)
                for half in (0, 1):
                    pb = 64 * half
                    nc.tensor.matmul(
                        pss[half][:, c0:c1],
                        kT_sb[pb:pb + 64, pair, q0k:q0k + 128],
                        qT_sb[pb:pb + 64, pair, lo + c0:lo + c1],
                        start=True, stop=True,
                    )
                c0 = c1
            for half, st in ((0, sta), (1, stb)):
                if dve_z or ti == 1:
                    # Z contribution via DVE row-sum (second chunks are
                    # <=512 wide; cheap) -- saves an ACT READ_ACCUMULATOR.
                    nc.scalar.activation(
                        e_rows[half][:, lo - q0k:lo - q0k + w], pss[half][:, 0:w],
                        Exp, scale=SCALE,
                    )
                    if not dve_z:
                        nc.vector.tensor_reduce(
                            st["zp"][:, kt, 1:2],
                            e_rows[half][:, lo - q0k:lo - q0k + w],
                            axis=mybir.AxisListType.X, op=mybir.AluOpType.add,
                        )
                else:
                    nc.scalar.activation(
                        e_rows[half][:, lo - q0k:lo - q0k + w], pss[half][:, 0:w],
                        Exp, scale=SCALE,
                        accum_out=st["zp"][:, kt, ti:ti + 1],
                    )
        for half, st in ((0, sta), (1, stb)):
            _diag_fix(st, kt, e_rows[half], dve_z)

    def z_v2(st, k0, k1):
        """finalize Z for rows [k0, k1) and scale V rows by 1/Z."""
        h = st["h"]
        n = k1 - k0
        zs = zpool.tile([128, n], F32, tag="zs", name="zs")
        nc.vector.tensor_reduce(zs, st["zp"][:, k0:k1, :],
                                axis=mybir.AxisListType.X,
                                op=mybir.AluOpType.add)
        zv = zpool.tile([128, n], F32, tag="zv", name="zv")
        nc.vector.tensor_sub(zv, zs, st["inv"][:, k0:k1])
        zi = zpool.tile([128, n], F32, tag="zi", name="zi")
        nc.vector.reciprocal(zi, zv)
        zia = zi[:, :]
        zi_bc = bass.AP(tensor=zia.tensor, offset=zia.offset,
                        ap=[zia.ap[0], zia.ap[1], [0, HD]])
        nc.vector.tensor_mul(
            v2_sb[:, k0:k1, HD * h:HD * h + HD],
            v_sb[:, k0:k1, HD * h:HD * h + HD],
            zi_bc,
        )

    def ctx_begin(sta, stb, qc):
        ps = small_ps.tile([128, 512], F32, tag="ps512", name="cpp")
        return {"a": sta, "b": stb, "qc": qc, "ps": ps,
                "n": 2 * (4 * qc + 4), "done": 0, "first_kt": None}

    def ctx_mms(ch, kts):
        """col-packed ctx matmuls for the given kt subset.  The FIRST call's
        first kt must cover the whole 512-col window (any kt <= 4qc does).
        start/stop must be set on BOTH halves' first/last matmuls (the two
        halves write disjoint psum partition ranges)."""
        qc = ch["qc"]
        if ch["first_kt"] is None:
            ch["first_kt"] = kts[0]
        for kt in kts:
            q0 = max(512 * qc, 128 * kt)
            w = 512 * qc + 512 - q0
            for half, st in ((0, ch["a"]), (1, ch["b"])):
                h = st["h"]
                rhs = st["e"][kt][:, q0 - 128 * kt:q0 - 128 * kt + w]
                ch["done"] += 1
                nc.tensor.matmul(
                    ch["ps"][64 * half:64 * half + 64, q0 - 512 * qc:512],
                    v2_sb[:, kt, HD * h:HD * h + HD],
                    rhs,
                    start=(kt == ch["first_kt"]),
                    stop=(ch["done"] >= ch["n"] - 1),
                    tile_position=(0, 64 * half),
                    skip_group_check=True,
                )
        assert ch["done"] <= ch["n"]

    def ctx_finish(ch):
        assert ch["done"] == ch["n"]
        pair = ch["a"]["h"] // 2
        qc = ch["qc"]
        nc.vector.tensor_copy(out_sb[:, pair, 512 * qc:512 * qc + 512],
                              ch["ps"])
        nc.sync.dma_start(
            out=out_ap[128 * pair:128 * pair + 128, 512 * qc:512 * qc + 512],
            in_=out_sb[:, pair, 512 * qc:512 * qc + 512],
        )

    def ctx_full(sta, stb, qc):
        ch = ctx_begin(sta, stb, qc)
        ctx_mms(ch, range(4 * qc + 4))
        ctx_finish(ch)

    # ---- emission (order = scheduling priority) ----
    # phase 0: pair-0 scores descending, with pair-0 q/k + v projections as
    # per-row PE filler; pair-1 q/k projections are spread through the
    # qc1/qc0 groups, whose big rows leave ACT far ahead of PE -- this
    # balances every window and ends phase 0 with pair-1 ready to score.
    st0 = alloc_head(0)
    st1 = alloc_head(1)
    st2 = alloc_head(2)
    st3 = alloc_head(3)
    fill1 = {3: [], 2: [],
             1: [("q", 1, 3), ("k", 1, 3), ("q", 1, 2), ("k", 1, 2)],
             0: [("q", 1, 1), ("k", 1, 1), ("q", 1, 0), ("k", 1, 0)]}
    for qc in (3, 2, 1, 0):
        proj_chain("q", 0, qc)
        proj_chain("k", 0, qc)
        fillers = list(fill1[qc])
        for kt in range(4 * qc + 3, 4 * qc - 1, -1):
            score_row_pair(st0, st1, kt)
            proj_v(kt)         # fills the PE gap while ACT drains the row
            if fillers:
                proj_chain(*fillers.pop(0))
        z_v2(st0, 4 * qc, 4 * qc + 4)
        z_v2(st1, 4 * qc, 4 * qc + 4)
    # phase 1: pair-1 scores descending.  Rows 15..8 flow immediately (4 E
    # slots); pair-0 ctx chains run once ACT has drained pair-0 (their z is
    # ready by then) and free E tags 4-7/0-3 in exactly the order the
    # remaining descending rows reuse them.  ch3's high-kt matmuls run as
    # filler before rows 3..0 exist; only kt 3..0 work remains for the tail.
    for kt in (15, 14, 13, 12):
        score_row_pair(st2, st3, kt)
    z_v2(st2, 12, 16)
    z_v2(st3, 12, 16)
    for kt in (11, 10, 9, 8):
        score_row_pair(st2, st3, kt)
    z_v2(st2, 8, 12)
    z_v2(st3, 8, 12)
    ctx_full(st0, st1, 3)
    ctx_full(st0, st1, 2)
    ctx_full(st0, st1, 1)
    for kt in (7, 6, 5, 4):
        score_row_pair(st2, st3, kt)
    z_v2(st2, 4, 8)
    z_v2(st3, 4, 8)
    ctx_full(st0, st1, 0)
    ch3 = ctx_begin(st2, st3, 3)
    ctx_mms(ch3, [12, 13, 14, 15, 11, 10, 9, 8, 7, 6, 5, 4])
    for kt in (3, 2, 1, 0):
        score_row_pair(st2, st3, kt)
    z_v2(st2, 0, 4)
    z_v2(st3, 0, 4)
    # tail
    ctx_mms(ch3, [3, 2, 1, 0])
    ctx_finish(ch3)
    ctx_full(st2, st3, 0)
    ctx_full(st2, st3, 1)
    ctx_full(st2, st3, 2)


_PROG = None


def _build_program():
    global _PROG
    if _PROG is not None:
        return _PROG
    nc = bacc.Bacc("TRN2", target_bir_lowering=False, debug=False,
                   num_devices=NCORES)
    xT = nc.dram_tensor("xT", [128, 4 * KC * 512], BF16, kind="ExternalInput").ap()
    wq = nc.dram_tensor("wq", [128, KC * HL * HD], BF16, kind="ExternalInput").ap()
    wk = nc.dram_tensor("wk", [128, KC * HL * HD], BF16, kind="ExternalInput").ap()
    wv = nc.dram_tensor("wv", [128, KC * HL * HD], BF16, kind="ExternalInput").ap()
    out = nc.dram_tensor("out", [HL * HD, S], F32, kind="ExternalOutput").ap()
    with tile.TileContext(nc) as tc:
        with ExitStack() as stack:
            _emit(stack, tc, out, xT, wq, wk, wv)
    nc.compile()
    _PROG = nc
    return nc


def _arrange_xT(xb):
    # [S, D] batch slice -> xT[d, s] -> [p, qc, c, j] with d = c*128+p,
    # s = 512*qc + j -> flattened [128, 4*8*512]; every DMA chunk
    # (one qc) is then contiguous per partition.
    xT = np.asarray(xb).T                              # [D, S]
    t = xT.reshape(KC, 128, QC, 512).transpose(1, 2, 0, 3)
    return np.ascontiguousarray(t.reshape(128, QC * KC * 512))


def _arrange_w(w, cols):
    # [D, 256] slice -> [p, c, n] with d = c*128+p -> [128, 8*256]
    t = np.asarray(w)[:, cols].reshape(KC, 128, HL * HD).transpose(1, 0, 2)
    return np.ascontiguousarray(t.reshape(128, KC * HL * HD))


def make_in_maps(x, Wq, Wk, Wv):
    bf = ml_dtypes.bfloat16
    in_maps = []
    for core in range(NCORES):
        b, g = divmod(core, NCORES // B)
        cols = slice(HL * HD * g, HL * HD * (g + 1))
        in_maps.append({
            "xT": _arrange_xT(x[b]).astype(bf),
            "wq": _arrange_w(Wq, cols).astype(bf),
            "wk": _arrange_w(Wk, cols).astype(bf),
            "wv": _arrange_w(Wv, cols).astype(bf),
        })
    return in_maps


def assemble(results):
    out = np.empty((B, S, H * HD), np.float32)
    for core in range(NCORES):
        b, g = divmod(core, NCORES // B)
        out[b, :, HL * HD * g:HL * HD * (g + 1)] = results[core]["out"].T
    return out


def kernel(**inputs):
    nc = _build_program()
    in_maps = make_in_maps(inputs["x"], inputs["Wq"], inputs["Wk"], inputs["Wv"])
    res = run_bass_kernel_spmd(nc, in_maps, list(range(NCORES)))
    return assemble(res.results)
